# revision 21
# baseline (speedup 1.0000x reference)
"""ASP layer (low-rank masked attention + residual layernorm) on 8 TRN2 cores.

Sharding: core c handles batch b = c // 2, query half h = c % 2.
Each core receives x/mask for its batch ROTATED so that its 1024 queries are
rows 0:1024 (keys are just permuted; softmax and delta are invariant to key
order). The device program is identical on all cores (SPMD); only data
differs.

Device math per core (N=2048 keys, Q=1024 queries, D=1024, R=64):
  QtKt = [U|V]^T @ x^T          (PE bf16, fp32 accum; x^T precomputed on host)
  Qt   = QtKt[0:64]   * (mask*s).T   (DVE; s = 1/sqrt(r_eff) folded on host)
  Kt   = QtKt[64:128] * mask.T       (DVE)
  St   = Kt_tile^T @ Qt         (PE; scores TRANSPOSED [k, q] so exp output
                                 is directly the delta stationary)
  Et   = exp(St - 3.5)          (ACT, psum -> sbuf FP8 e4m3; softmax and the
                                 rs-scaled LN are shift-invariant.)
  rs   = ones^T @ Et            (PE fp8 DoubleRow; softmax row sums)
  delta= Et^T @ x8              (PE fp8 DoubleRow, fp32 accum)
  z    = rs*x_q + delta         (DVE; LN is scale-invariant per token)
  out  = LN(z)                  (DVE moments + single ACT sqrt)
gamma/beta are applied on the host.

Schedule (v2): the whole kernel is one software-pipelined PE stream.
  - All input DMAs are issued up front on 4 HWDGE rings (SP/ACT/Pool/DVE)
    in consumption order: uv, mt, ident, x^T c0, c1, x8 kt0-3, c2, kt4-7,
    c3, kt8-15, xq. ~6.75MB of critical bytes => first delta possible ~17us.
  - Warmup spin sized to end when uv+mt+x^T c0 land (opens the PE clock
    gate and hides the initial DMA latency).
  - Projections interleave qc0 scores+rowsums; the first delta pair (qb0/1)
    runs chain-major right after the projections with qc1 scores + qc1
    rowsums + both rowsum transposes as fillers; pairs (2,3),(4,5),(6,7)
    then stream clean with 6 PSUM banks so only qb7's epilogue trails the
    final matmul.
  - Tail: the tile drain's semaphore waits are distributed across PE/ACT/SP
    in parallel (instead of ~11 serial NOPs on SP), DVE/Pool gate on a flag
    semaphore, and Pool alone resets+clears the tile semaphore range. No
    full engine barriers at the end.
"""

import os
import sys

sys.path.insert(0, "/opt/trn_rl_repo")

import numpy as np
import ml_dtypes

B, N, D, R = 4, 2048, 1024, 64
NCORES = 8
Q = N // 2          # queries per core
NQB = Q // 128      # query blocks per core
NKT = N // 128      # key tiles
NDT = D // 128      # d tiles
LN_EPS = 1e-5
WARMUP_MM = 75      # PE spin sized to end as uv+mt+x^T cols 0:1024 land
EXP_SHIFT = -3.5    # exp(s + EXP_SHIFT): keeps Et below the e4m3 max of 240

BF16 = ml_dtypes.bfloat16
FP8 = ml_dtypes.float8_e4m3

_CACHE = {}


def _split_waits(nc, max_waits=1):
    """walrus in this container rejects instructions carrying more than ~1
    sem-wait (e.g. Drain/CTRL and the XPOSE DMA encodings). Move excess waits
    onto injected same-engine nops that precede the instruction — engines are
    program-ordered, so semantics are unchanged."""
    from concourse import mybir

    n = 0
    for fn in nc.m.functions:
        for bb in fn.blocks:
            insts = bb.instructions
            new_list = []
            for inst in insts:
                si = inst.sync_info
                waits = list(si.on_wait) if si and si.on_wait else []
                if len(waits) > max_waits:
                    excess = waits[: -max_waits]
                    si.on_wait = waits[-max_waits:]
                    for w in excess:
                        nop = mybir.InstNoOp(name=f"I-wsplit-{n}", ins=[],
                                             outs=[])
                        n += 1
                        nop.engine = inst.engine
                        nop.sync_info = mybir.SyncInfo(on_wait=[w],
                                                       on_update=[])
                        nc.register_instruction(nop)
                        new_list.append(nop)
                new_list.append(inst)
            insts[:] = new_list


def _patch_tile_drain():
    import concourse.tile as tile
    from concourse import mybir
    from concourse.vector_clock import ScopedClock

    if getattr(tile.TileContext, "_drain_patched", False):
        return

    def _drain_and_barrier(self, tick_clock, wait_clock):
        nc = self.nc
        # Collect the full end-of-kernel wait set on the sync drain, then
        # redistribute it: one wait per NOP, round-robin across PE/ACT/SP so
        # the ~600ns-per-sem-check cost is paid in parallel instead of as a
        # serial chain on SP. DVE and Pool (whose end-of-NEFF semaphore-wipe
        # chunks contain the live tile sems) gate on a flag incremented by
        # the three waiting engines; Pool then resets the tile sem range for
        # re-execution. No full engine barriers.
        drain_inst = nc.sync.drain()
        wait_clock.add_sem_waits(
            drain_inst.ins, ScopedClock({None: tick_clock.global_clock})
        )
        assert self.sems is not None
        popped = nc._tile_sem_poison_stack.pop()
        assert popped is self._sem_poison

        si = drain_inst.ins.sync_info
        waits = list(si.on_wait) if si and si.on_wait else []
        si.on_wait = []

        sem_nums = sorted(
            s.num if hasattr(s, "num") else s
            for s in self.sems.allocated().values()
        )
        flag = nc.alloc_semaphore("tail_flag")

        wait_engines = [nc.tensor, nc.scalar, nc.sync]
        for i, w in enumerate(waits):
            eng = wait_engines[i % len(wait_engines)]
            nop = eng.nop()
            nop.ins.sync_info = mybir.SyncInfo(on_wait=[w], on_update=[])
        for eng in wait_engines:
            eng.sem_inc(flag, 1)
        nc.vector.wait_ge(flag, len(wait_engines))
        nc.gpsimd.wait_ge(flag, len(wait_engines))
        if sem_nums:
            lo, hi = min(sem_nums), max(sem_nums)
            rng = range(lo, max(hi, flag.num) + 1)
            nc.gpsimd.dma_reset(rng)
            nc.gpsimd.sem_clear(rng)
        _split_waits(nc)

    tile.TileContext._drain_and_barrier = _drain_and_barrier
    tile.TileContext._drain_patched = True


def build_program():
    import contextlib

    import concourse.bass as bass
    import concourse.tile as tile
    from concourse import mybir

    _patch_tile_drain()
    f32 = mybir.dt.float32
    bf16 = mybir.dt.bfloat16
    fp8 = mybir.dt.float8e4
    AF = mybir.ActivationFunctionType
    DR = mybir.MatmulPerfMode.DoubleRow

    nc = bass.Bass("TRN2", target_bir_lowering=False, debug=False,
                   num_devices=NCORES)

    # x8/xt/xq are stored PARTITION-MAJOR in dram (host pre-shuffle): dram
    # bytes for partition p are contiguous, so every DMA line is a 2-4KB
    # linear run instead of 1KB (descriptor-rate-bound DMA runs ~2-4x
    # faster per queue).
    x8_d = nc.dram_tensor("x8", [128, NKT, D], fp8, kind="ExternalInput").ap()
    xt_d = nc.dram_tensor("xt", [128, NDT, N], bf16,
                          kind="ExternalInput").ap()
    xq_d = nc.dram_tensor("xq", [128, NQB, D], bf16,
                          kind="ExternalInput").ap()
    mt_d = nc.dram_tensor("mt", [2 * R, N], bf16, kind="ExternalInput").ap()
    uv_d = nc.dram_tensor("uv", [128, NDT, 2 * R], bf16,
                          kind="ExternalInput").ap()
    id_d = nc.dram_tensor("ident", [128, 128], f32, kind="ExternalInput").ap()
    out_d = nc.dram_tensor("out", [Q, D], bf16, kind="ExternalOutput").ap()

    with tile.TileContext(nc) as tc:
        with contextlib.ExitStack() as ctx:
            const = ctx.enter_context(tc.tile_pool(name="const", bufs=1))
            eps_sb = const.tile([128, 1], f32)
            shift_sb = const.tile([128, 1], f32)
            ones_sb = const.tile([128, 2, 128], fp8)
            warm_sb = const.tile([128, 128], bf16)
            uv_sb = const.tile([128, NDT, 2 * R], bf16)
            xt_sb = const.tile([128, NDT, N], bf16)
            mt_sb = const.tile([2 * R, N], bf16)
            x8_sb = const.tile([128, NKT, D], fp8)
            xq_sb = const.tile([128, NQB, D], bf16)
            id_sb = const.tile([128, 128], f32)
            qt_sb = const.tile([R, Q], bf16)
            kt_sb = const.tile([R, N], bf16)
            # Et layout: [p, qc, t(=kt pair), h, 512] — h indexes the kt pair
            # so [:, qc, t] is a ready-made [128, 2, 512] DoubleRow operand
            et_sb = const.tile([128, 2, NKT // 2, 2, 512], fp8)
            # throwaway Square output (only its accum_out matters); same-
            # engine WAW ordering makes sharing one buffer safe
            sq_scr = const.tile([128, 512], f32)

            # warm_sb first: the PE warmup spin waits only on this memset
            nc.vector.memset(warm_sb, 0.5)
            nc.vector.memset(ones_sb, 1.0)
            nc.vector.memset(eps_sb, LN_EPS)
            nc.vector.memset(shift_sb, EXP_SHIFT)
            # preload both ACT function tables during the DMA-wait window so
            # no 1.3us ACT_TABLE_LOAD ever lands mid-stream
            tbl_scr = const.tile([128, 1], f32)
            nc.scalar.activation(out=tbl_scr, in_=eps_sb, func=AF.Exp)
            nc.scalar.activation(out=tbl_scr, in_=eps_sb, func=AF.Sqrt)

            # ---- all input DMAs up front. Each HWDGE ring fans out over ~4
            # hw queues with one outstanding transfer per queue, so a ring's
            # k-th..(k+3)-th transfers run concurrently and round k+1 waits
            # for round k. scalar (ACT) only carries transfers it can issue
            # before the first exp (~10us); sync carries the late rounds;
            # gpsimd's software-DGE queue takes the small early constants.
            def xtA(ring, dt):   # x^T columns 0:1024 of d-tile dt (256KB)
                ring.dma_start(out=xt_sb[:, dt, 0:1024],
                               in_=xt_d[:, dt, 0:1024])

            def xtB(ring, dt):   # x^T columns 1024:2048
                ring.dma_start(out=xt_sb[:, dt, 1024:2048],
                               in_=xt_d[:, dt, 1024:2048])

            def x8q(ring, q):    # x8 quad: key tiles 4q..4q+3 (512KB)
                ring.dma_start(out=x8_sb[:, 4 * q:4 * q + 4, :],
                               in_=x8_d[:, 4 * q:4 * q + 4, :])

            # gpsimd (software ring, starts ~2us late): small constants
            # first, then its share of the bulk.
            nc.gpsimd.dma_start(out=uv_sb, in_=uv_d)
            nc.gpsimd.dma_start(out=mt_sb, in_=mt_d)
            for dt in (6, 7):
                xtB(nc.gpsimd, dt)
            nc.gpsimd.dma_start(out=id_sb, in_=id_d)
            x8q(nc.gpsimd, 3)
            # scalar (ACT) carries only what it can issue before the first
            # exp; sync carries the rest including all the late rounds.
            for dt in (1, 3, 5):
                xtA(nc.scalar, dt)
            for dt in (1, 3, 5):
                xtB(nc.scalar, dt)
            x8q(nc.scalar, 2)
            for dt in (0, 2, 4, 6, 7):
                xtA(nc.sync, dt)
            for dt in (0, 2, 4):
                xtB(nc.sync, dt)
            x8q(nc.sync, 0)
            x8q(nc.sync, 1)
            for b in range(0, NQB, 2):
                nc.sync.dma_start(out=xq_sb[:, b:b + 2, :],
                                  in_=xq_d[:, b:b + 2, :])

            # ---- pools ----
            work = ctx.enter_context(tc.tile_pool(name="work", bufs=2))
            keep = ctx.enter_context(tc.tile_pool(name="keep", bufs=1))
            small = ctx.enter_context(tc.tile_pool(name="small", bufs=3))
            rsq_sb = keep.tile([128, NQB], f32)   # softmax rowsums, [q,1]/qb

            # PSUM budget (8 banks): phase 0: ps0(2) + st(2x2) + rr(1) = 7;
            # phase 1 (projections done): d(3) + st(4) + rr(1) = 8;
            # phase 2 (scores+rowsums done): d(6).
            phaseA = ctx.enter_context(contextlib.ExitStack())
            st_pool = phaseA.enter_context(
                tc.tile_pool(name="st_ps", bufs=2, space="PSUM"))
            rr_pool = phaseA.enter_context(
                tc.tile_pool(name="rr_ps", bufs=1, space="PSUM"))

            def st_pair(qc, t):
                """St = Kt_kt^T @ Qt_qc for kt pair (2t, 2t+1); Et = exp."""
                qlo = qc * 512
                st_ps = st_pool.tile([128, 2, 512], f32,
                                     name=f"st_{qc}_{t}", tag="st")
                for h in range(2):
                    kt = 2 * t + h
                    nc.tensor.matmul(
                        st_ps[:, h],
                        kt_sb[:, kt * 128:(kt + 1) * 128],
                        qt_sb[:, qlo:qlo + 512],
                        start=True, stop=True,
                    )
                nc.scalar.activation(out=et_sb[:, qc, t], in_=st_ps,
                                     func=AF.Exp, bias=shift_sb)

            def rs_mm(qc, t, rr_ps):
                """one accumulating DoubleRow step of rs = ones^T @ Et; the
                all-ones stationary is [128, 2, 128] (M=1 fails the walrus
                ldweights ISA check), so every psum partition receives the
                same rowsum row — rs_fix reads row 0."""
                nc.tensor.matmul(
                    rr_ps, ones_sb,
                    et_sb[:, qc, t],
                    start=(t == 0), stop=(t == NKT // 2 - 1),
                    perf_mode=DR,
                )

            def rs_fix(qc, rr_ps):
                """rowsums psum -> sbuf, then layout fix [1,q] -> [q,1] per
                query block via tiny PE transposes sharing the rr bank."""
                rs_sb = small.tile([1, 512], f32, tag="rs_sb")
                nc.vector.tensor_copy(rs_sb, rr_ps[0:1, :])
                for j in range(4):
                    qb = qc * 4 + j
                    nc.tensor.transpose(rr_ps[:, j:j + 1],
                                        rs_sb[0:1, j * 128:(j + 1) * 128],
                                        id_sb[0:1, 0:1])
                    nc.vector.tensor_copy(rsq_sb[:, qb:qb + 1],
                                          rr_ps[:, j:j + 1])

            def epi_half(qb, d_ps, dc, y, zs, zss):
                """z half: rs*x_q + delta for 512 features. sum(z) rides the
                same DVE op via accum_out; sum(z^2) goes to the idle ACT as
                Square+accum."""
                lo, hi = dc * 512, (dc + 1) * 512
                nc.vector.scalar_tensor_tensor(
                    out=y[:, lo:hi], in0=xq_sb[:, qb, lo:hi],
                    scalar=rsq_sb[:, qb:qb + 1], in1=d_ps,
                    op0=mybir.AluOpType.mult, op1=mybir.AluOpType.add,
                    accum_out=zs[:, dc:dc + 1],
                )
                nc.scalar.activation(out=sq_scr, in_=y[:, lo:hi],
                                     func=AF.Square,
                                     accum_out=zss[:, dc:dc + 1])

            def epi_finish(qb, y, zs, zss, last=False):
                """out = LN(z) from the accumulated moments:
                var = (sum(z^2) - sum(z)^2/D) / D; out = z*rstd - mean*rstd.
                Split per dc half so each half's store DMA starts early."""
                t1 = small.tile([128, 1], f32, tag="t1")
                nc.vector.tensor_add(t1, zs[:, 0:1], zs[:, 1:2])
                dv = small.tile([128, 1], f32, tag="dv")
                nc.vector.scalar_tensor_tensor(
                    out=dv, in0=t1, scalar=1.0 / D, in1=t1,
                    op0=mybir.AluOpType.mult, op1=mybir.AluOpType.mult)
                t2 = small.tile([128, 1], f32, tag="t2")
                nc.vector.tensor_add(t2, zss[:, 0:1], zss[:, 1:2])
                vv = small.tile([128, 1], f32, tag="vv")
                nc.vector.tensor_sub(vv, t2, dv)
                sd = small.tile([128, 1], f32, tag="sd")
                nc.scalar.activation(out=sd, in_=vv, func=AF.Sqrt,
                                     scale=1.0 / D, bias=eps_sb)
                rstd = small.tile([128, 1], f32, tag="rstd")
                nc.vector.reciprocal(rstd, sd)
                mr = small.tile([128, 1], f32, tag="mr")
                nc.vector.scalar_tensor_tensor(
                    out=mr, in0=t1, scalar=1.0 / D, in1=rstd,
                    op0=mybir.AluOpType.mult, op1=mybir.AluOpType.mult)
                o_sb = work.tile([128, D], bf16, tag="o")
                # store halves on alternating rings; the final block's store
                # is split into quarters across 4 queues so the last (tail-
                # critical) transfer is 4x shorter.
                store_rings = [nc.sync, nc.scalar]
                n_pieces = 4 if last else 2
                w = D // n_pieces
                for pc in range(n_pieces):
                    lo, hi = pc * w, (pc + 1) * w
                    nc.vector.tensor_scalar(out=o_sb[:, lo:hi],
                                            in0=y[:, lo:hi],
                                            scalar1=rstd, scalar2=mr,
                                            op0=mybir.AluOpType.mult,
                                            op1=mybir.AluOpType.subtract)
                    store_rings[pc % 2].dma_start(
                        out=out_d[qb * 128:(qb + 1) * 128, lo:hi],
                        in_=o_sb[:, lo:hi])

            def make_chain(qb, dc, d_pool, y, zs, zss):
                """8 accumulating delta matmul thunks for one (qb, dc) half;
                the stop matmul issues the half's DVE/ACT epilogue inline."""
                qc, j = divmod(qb, 4)
                d_ps = d_pool.tile([128, 512], f32, name=f"d_{qb}_{dc}",
                                   tag="d")
                mms = []
                for t in range(NKT // 2):
                    def mm(t=t):
                        nc.tensor.matmul(
                            d_ps,
                            et_sb[:, qc, t, :, j * 128:(j + 1) * 128],
                            x8_sb[:, 2 * t:2 * t + 2,
                                  dc * 512:(dc + 1) * 512],
                            start=(t == 0), stop=(t == NKT // 2 - 1),
                            perf_mode=DR,
                        )
                        if t == NKT // 2 - 1:
                            epi_half(qb, d_ps, dc, y, zs, zss)
                    mms.append(mm)
                return mms

            def qb_bufs(qb):
                y = work.tile([128, D], f32, tag="y")
                zs = small.tile([128, 2], f32, tag="zs")
                zss = small.tile([128, 2], f32, tag="zss")
                return y, zs, zss

            # ---- phase 0: warmup spin + projections + qc0 scores ----
            rr0 = rr_pool.tile([128, 512], f32, name="rr_0", tag="rr")
            with tc.tile_pool(name="warm", bufs=1, space="PSUM") as warm:
                w_ps = warm.tile([128, 128], f32)
                for _ in range(WARMUP_MM):
                    nc.tensor.matmul(w_ps, warm_sb, warm_sb,
                                     start=True, stop=True)
            with tc.tile_pool(name="ps0", bufs=2, space="PSUM") as ps0:
                # proj chunk nch, then qc0 score/rowsum fillers between
                # chunks so the PE stream stays dense while DMA streams in.
                # The d-tile accumulation order follows the expected DMA
                # arrival order across the three rings.
                DT_ORDER = (0, 2, 1, 4, 3, 6, 5, 7)
                for nch in range(4):
                    lo, hi = nch * 512, (nch + 1) * 512
                    qk_ps = ps0.tile([128, 512], f32)
                    for i, dt in enumerate(DT_ORDER):
                        nc.tensor.matmul(
                            qk_ps, uv_sb[:, dt, :],
                            xt_sb[:, dt, lo:hi],
                            start=(i == 0), stop=(i == NDT - 1),
                        )
                    if lo < Q:
                        nc.vector.tensor_mul(qt_sb[:, lo:hi],
                                             qk_ps[0:R, :], mt_sb[0:R, lo:hi])
                    nc.vector.tensor_mul(kt_sb[:, lo:hi],
                                         qk_ps[R:2 * R, :],
                                         mt_sb[R:2 * R, lo:hi])
                    if nch >= 1:
                        st_pair(0, 2 * (nch - 1))
                        st_pair(0, 2 * (nch - 1) + 1)
                    if nch >= 2:
                        for t in range(2 * (nch - 2), 2 * (nch - 1)):
                            rs_mm(0, t, rr0)
                        # early qc1 score pairs: ACT is idle here, so their
                        # exps clear long before the rowsum chain needs them
                        st_pair(1, nch - 2)

            # ---- phase 1: finish qc0 scores/rowsums, first delta pair
            # (qb0, qb1) chain-major with qc1 scores + rowsums as fillers ----
            d3 = phaseA.enter_context(
                tc.tile_pool(name="d_ps", bufs=3, space="PSUM"))
            st_pair(0, 6)
            st_pair(0, 7)
            rs_mm(0, 4, rr0)
            st_pair(1, 2)
            st_pair(1, 3)
            rs_mm(0, 5, rr0)

            y0, zs0, zss0 = qb_bufs(0)
            y1, zs1, zss1 = qb_bufs(1)
            a0 = make_chain(0, 0, d3, y0, zs0, zss0)
            b0 = make_chain(1, 0, d3, y1, zs1, zss1)
            rr1 = rr_pool.tile([128, 512], f32, name="rr_1", tag="rr")

            # A0: qb0.dc0 with qc1 score fillers; qc0 rowsum chain finishes
            # and is transposed before A0's stop so epi(0,0) has rsq ready.
            a0[0](); a0[1]()
            st_pair(1, 4)
            a0[2](); a0[3]()
            st_pair(1, 5)
            a0[4](); a0[5]()
            rs_mm(0, 6, rr0)
            a0[6]()
            rs_mm(0, 7, rr0)
            rs_fix(0, rr0)
            a0[7]()
            # B0: qb1.dc0 carries the last qc1 score pairs
            b0[0](); b0[1]()
            st_pair(1, 6)
            b0[2](); b0[3]()
            st_pair(1, 7)
            b0[4](); b0[5]()
            rs_mm(1, 0, rr1)
            b0[6](); b0[7]()
            # A1: qb0.dc1 with the qc1 rowsum chain
            a1 = make_chain(0, 1, d3, y0, zs0, zss0)
            a1[0]()
            rs_mm(1, 1, rr1)
            a1[1](); a1[2]()
            rs_mm(1, 2, rr1)
            a1[3](); a1[4]()
            rs_mm(1, 3, rr1)
            a1[5](); a1[6]()
            rs_mm(1, 4, rr1)
            a1[7]()
            epi_finish(0, y0, zs0, zss0)
            # B1: qb1.dc1 with the tail of the qc1 rowsum chain
            b1 = make_chain(1, 1, d3, y1, zs1, zss1)
            b1[0]()
            rs_mm(1, 5, rr1)
            b1[1]()
            rs_mm(1, 6, rr1)
            b1[2](); b1[3]()
            rs_mm(1, 7, rr1)
            b1[4]()
            rs_fix(1, rr1)
            b1[5](); b1[6](); b1[7]()
            epi_finish(1, y1, zs1, zss1)

            # ---- phase 2: remaining pairs, 6 PSUM banks, pure deltas ----
            phaseA.close()
            d6 = ctx.enter_context(
                tc.tile_pool(name="d_ps_b", bufs=6, space="PSUM"))
            for qa in range(2, NQB, 2):
                ya, zsa, zssa = qb_bufs(qa)
                yb, zsb, zssb = qb_bufs(qa + 1)
                ca0 = make_chain(qa, 0, d6, ya, zsa, zssa)
                cb0 = make_chain(qa + 1, 0, d6, yb, zsb, zssb)
                for mm in ca0 + cb0:
                    mm()
                ca1 = make_chain(qa, 1, d6, ya, zsa, zssa)
                for mm in ca1:
                    mm()
                epi_finish(qa, ya, zsa, zssa)
                cb1 = make_chain(qa + 1, 1, d6, yb, zsb, zssb)
                for mm in cb1:
                    mm()
                epi_finish(qa + 1, yb, zsb, zssb, last=(qa + 1 == NQB - 1))

    return nc


def prep_core_inputs(x, mask, U, V):
    """Per-core input dicts (host-side sharding/layout prep)."""
    # [D, 2R] -> [128, NDT, 2R]: partition-major so the device DMA is one
    # contiguous 2KB-per-partition read
    uv = np.ascontiguousarray(
        np.concatenate([U, V], axis=1).astype(BF16)
        .reshape(NDT, 128, 2 * R).transpose(1, 0, 2))
    ident = np.eye(128, dtype=np.float32)
    ins = []
    for c in range(NCORES):
        b, h = divmod(c, 2)
        rot = np.roll(np.arange(N), -h * Q)
        xr = np.ascontiguousarray(x[b][rot])            # [N, D] f32
        mr = np.ascontiguousarray(mask[b][rot])         # [N, R] f32
        s = 1.0 / np.sqrt(np.maximum(mr.sum(axis=1), 1.0))   # [N]
        mq = (mr * s[:, None]).astype(BF16).T           # [R, N]
        mk = mr.astype(BF16).T                          # [R, N]
        xbf = xr.astype(BF16)
        # partition-major dram layouts: [...] -> [128, tiles, free] so each
        # partition's dram bytes are one contiguous run (big DMA lines)
        x8p = np.ascontiguousarray(
            xr.astype(FP8).reshape(NKT, 128, D).transpose(1, 0, 2))
        xtp = np.ascontiguousarray(
            xbf.T.reshape(NDT, 128, N).transpose(1, 0, 2))
        xqp = np.ascontiguousarray(
            xbf[:Q].reshape(NQB, 128, D).transpose(1, 0, 2))
        ins.append({
            "x8": x8p,
            "xt": xtp,
            "xq": xqp,
            "mt": np.ascontiguousarray(np.concatenate([mq, mk], axis=0)),
            "uv": uv,
            "ident": ident,
        })
    return ins


WALRUS_MAX_SEM = 176    # the NEFF exit routine wipes semaphores 0..max in
                        # ~51-per-engine serial chunks (~6us); our program
                        # tops out at sem ~170, so cap the wipe there.


def _patch_walrus_maxsem():
    if not WALRUS_MAX_SEM:
        return
    import concourse.bass_utils as bu

    if getattr(bu, "_asp_walrus_shim", None):
        return
    real = bu.get_walrus_driver()
    shim = f"/tmp/asp_walrus_shim_{WALRUS_MAX_SEM}.sh"
    with open(shim, "w") as f:
        f.write(f'#!/bin/sh\nexec {real} "$@" '
                f'--max-sem-num={WALRUS_MAX_SEM}\n')
    os.chmod(shim, 0o755)
    bu.get_walrus_driver = lambda: shim
    bu._asp_walrus_shim = shim


def run_cores(ins, trace=False, trace_kwargs=None):
    from concourse.bass_utils import run_bass_kernel_spmd

    _patch_walrus_maxsem()
    if "nc" not in _CACHE:
        _CACHE["nc"] = build_program()
    kw = {}
    if trace:
        kw["trace"] = True
        kw.update(trace_kwargs or {})
    return run_bass_kernel_spmd(_CACHE["nc"], ins, list(range(NCORES)), **kw)


def kernel(x, mask, U, V, gamma, beta):
    x = np.asarray(x, dtype=np.float32)
    mask = np.asarray(mask, dtype=np.float32)
    U = np.asarray(U, dtype=np.float32)
    V = np.asarray(V, dtype=np.float32)
    gamma = np.asarray(gamma, dtype=np.float32)
    beta = np.asarray(beta, dtype=np.float32)

    ins = prep_core_inputs(x, mask, U, V)
    res = run_cores(ins)
    out = np.empty((B, N, D), dtype=np.float32)
    for c in range(NCORES):
        b, h = divmod(c, 2)
        out[b, h * Q:(h + 1) * Q] = res.results[c]["out"].astype(np.float32)
    return out * gamma + beta
